# revision 35
# baseline (speedup 1.0000x reference)
"""Trainium2 Bass kernel for nn_CrossAttention_37718402794152.

Head-parallel sharding across 8 NeuronCores: core h computes head h of both
"fundamental" bilinear attention outputs (dual-softmax cross attention), plus
its per-head slice of the final projection; the host sums the 8 partial
projections and adds the bias.

Per core / head-task (q from one input, k,v from the other):
  a = (q k^T) * scale        (PE, bf16, row-packed pairs into Wa/Wb PSUM)
  E = exp(a)                 (ACT, 5 chunks/strip: 4x1024 + 704, accum -> R)
  E2 = E*E                   (DVE, split 4096 + 704-tail passes)
  C += colsum(E)             (PE: per-128-window stationary-trick matmuls)
  P[c,m] += (vc[n,c]/R[n]) E2[n,m]   (PE, col-tiled pairs, 4 PSUM banks for
                              m<4096; the 704-wide m-tail is accumulated at
                              task end from persisted E2-tail slices)
  f[c,d] = sum_m P[c,m]/C[m] vc[m,d] (PE, via P-window transposes; 1/C is
                              folded into the transpose evacuation copies)
  out_h  = f^T-projected slice       (PE)

since softmax(a,-1)*softmax(a,-2) = exp(2a) / (rowsum(exp a) * colsum(exp a)).

The scalar engine (exp) is the critical path: per strip it runs 5 ACTIVATE
instructions (4800 payload cycles + 5x~310 fixed) + 5 accumulator reads.
The task boundary is overlapped: task-1's first two strips are emitted
before task-0's finalize so the ACT pipeline never drains mid-kernel.
"""

from collections import deque

import numpy as np
import ml_dtypes

import concourse.bass as bass
import concourse.mybir as mybir
import concourse.tile as tile
from concourse import bacc
from concourse.bass_utils import run_bass_kernel_spmd
from concourse.masks import make_identity

F32 = mybir.dt.float32
BF16 = mybir.dt.bfloat16
AF = mybir.ActivationFunctionType
ALU = mybir.AluOpType
AXL = mybir.AxisListType

# Problem constants (hardcoded; kernel.py must be self-contained).
N = 4800            # tokens
C = 256             # model dim
H = 8               # heads
HD = 32             # head dim
D = HD + 6          # 38: v + 6 positional features
SCALE = HD ** -0.5
H_IMG, W_IMG = 60, 80
_FX_N = (517.0 / 9.0) / 80.0 * 2.0
_FY_N = (517.0 / 8.0) / 60.0 * 2.0

NCH = (N + 127) // 128          # 38 n-strips of 128 (last = 64)
LASTP = N - (NCH - 1) * 128     # 64
MC = 512                        # P accumulation m-chunk (one PSUM bank)
NPB = 4                         # P PSUM banks (m < 4096 pair-packed)
MMAIN = NPB * 2 * MC            # 4096
MTAIL = N - MMAIN               # 704 (deferred-tail m columns)
# score chunks per strip: (offset, width); alternate lead/follow buffers
ACHUNKS = [(0, 1024), (1024, 1024), (2048, 1024), (3072, 1024), (4096, 704)]
NOVL = 3          # task-1 strips emitted before task-0's finalize
# Zero-accumulate filler matmuls per strip: saturating the PE flips the HAM
# clock gate to 8/8 (2.4 GHz) after one cold strip and HOLDS it there, which
# halves the real PE work and leaves ACT (exp) as the only critical path.
# Fewer fillers while prep jobs still trickle through the strips.
FILL_STEADY = [3, 3, 2, 3, 2]   # after each chunk's QK
FILL_TRICKLE = [2, 2, 1, 2, 1]


def _pn(i):
    return 128 if i < NCH - 1 else LASTP


def build_kernel(nc: bass.Bass, reps: int = 1):
    x1 = nc.dram_tensor("x1", [N, C], BF16, kind="ExternalInput").ap()
    x2 = nc.dram_tensor("x2", [N, C], BF16, kind="ExternalInput").ap()
    # per-head weights, host-prepped layouts (see kernel()):
    wq4 = nc.dram_tensor("wq4", [128, 2 * 128], BF16, kind="ExternalInput").ap()
    wk4 = nc.dram_tensor("wk4", [128, 2 * 128], BF16, kind="ExternalInput").ap()
    wv = nc.dram_tensor("wv", [128, 2 * HD], BF16, kind="ExternalInput").ap()
    pwt = nc.dram_tensor("pwt", [D, C], F32, kind="ExternalInput").ap()
    posb = nc.dram_tensor("posb", [128, NCH * 6], BF16, kind="ExternalInput").ap()
    out = nc.dram_tensor("out", [2, D, C], F32, kind="ExternalOutput").ap()

    with tile.TileContext(nc) as tc:
        for _ in range(reps):
            _tile_kernel(tc, out, x1, x2, wq4, wk4, wv, pwt, posb)
    return nc


class _Pools:
    pass


class StripMachine:
    """Emits the strip loop for one head-task with lag-1 colsum and lag-2 P
    accumulation so the PE never head-of-line-blocks the score refills."""

    def __init__(self, tc, P, task, qt, kt, vc, defer_pzero):
        self.tc = tc
        self.P = P
        self.nc = tc.nc
        self.task = task
        self.qt = qt
        self.kt = kt
        self.vc = vc
        self.defer_pzero = defer_pzero
        self.pzero_done = False
        self.first_cs = True
        self.fill_rr = 0
        nc = self.nc
        # P-bank tiles are created at first use (same tags => same banks;
        # creating them in first-use order keeps the tag version chain
        # consistent with emission order across the task boundary).
        self.p_tiles = None
        if not defer_pzero:
            self._make_p_tiles()
        # column-sum accumulator: c_sb[j, g] = sum_n E[n, g*128+j]
        self.c_sb = P.fin_pool.tile(
            [128, NCH], F32, tag=f"csb{task}", name=f"csb_{task}"
        )
        # per-strip 1/R-scaled vc, persisted for the deferred tail-P
        self.vcr_all = P.fin_pool.tile(
            [128, NCH * D], BF16, tag=f"vcr{task}", name=f"vcr_{task}"
        )
        self.prev = None        # (e_strip, pn, lead) of strip i-1
        self.pqueue = []        # [(strip, e2a, vcr_slice, pn), ...] lag-2

    def _make_p_tiles(self):
        self.p_tiles = [
            self.P.ppool.tile(
                [128, MC], F32, tag=f"p{b}", name=f"p{b}_t{self.task}"
            )
            for b in range(NPB)
        ]
        for b in range(NPB):
            self.nc.vector.memset(self.p_tiles[b][:], 0.0)
        self.pzero_done = True

    def tail_slot(self, s):
        # task-1 strips 0-2 are emitted before task-0's finalize (which reads
        # slots 0..37), so they get the three spare slots.
        return s if (self.task == 0 or s >= NOVL) else NCH + s

    def _emit_colsum(self, first, target):
        # target: the W buffer NOT being ACT-read at emission time (cols 0:38
        # are refilled by the next QK chunk into it, after the evacuation).
        nc = self.nc
        e_prev, ppn, _ = self.prev
        for g in range(NCH):
            gw = _pn(g)
            nc.tensor.matmul(
                target[:gw, g : g + 1],
                e_prev[:ppn, g * 128 : g * 128 + gw],
                self.P.ones_col[:ppn, :],
                start=True,
                stop=True,
            )
        if first:
            nc.vector.tensor_copy(self.c_sb[:, :], target[:, 0:NCH])
        else:
            nc.vector.tensor_add(
                self.c_sb[:, :], self.c_sb[:, :], target[:, 0:NCH]
            )

    def _emit_fill(self, n):
        # PE-saturating zero-adds (value-neutral: lhsT is all zeros)
        nc = self.nc
        P = self.P
        if self.p_tiles is None:
            return
        for _ in range(n):
            nc.tensor.matmul(
                self.p_tiles[self.fill_rr % NPB][0:D, 0:MC],
                P.z38[:, :],
                P.fillrhs[:, :],
                start=False,
                stop=False,
                tile_position=(0, 0),
                skip_group_check=True,
            )
            self.fill_rr += 1

    def _emit_p(self):
        nc = self.nc
        _, e2a, vcr, pn = self.pqueue.pop(0)
        for pc in range(NPB):
            mc0, mc1 = 2 * pc, 2 * pc + 1
            nc.tensor.matmul(
                self.p_tiles[pc][0:D, 0:MC],
                vcr[:pn, :],
                e2a[:pn, mc0 * MC : (mc0 + 1) * MC],
                start=False,
                stop=False,
                tile_position=(0, 0),
                skip_group_check=True,
            )
            nc.tensor.matmul(
                self.p_tiles[pc][64 : 64 + D, 0:MC],
                vcr[:pn, :],
                e2a[:pn, mc1 * MC : (mc1 + 1) * MC],
                start=False,
                stop=False,
                tile_position=(0, 64),
                skip_group_check=True,
            )

    def strip(self, i):
        nc = self.nc
        P = self.P
        if self.defer_pzero and not self.pzero_done and i >= NOVL:
            self._make_p_tiles()
        pn = _pn(i)
        lead = P.wa if i % 2 == 0 else P.wb
        follow = P.wb if i % 2 == 0 else P.wa
        e_strip = P.e_pool.tile([128, N], BF16, tag="e")
        rpart = P.small_pool.tile([128, 5], F32, tag="rpart")

        for k, (off, w) in enumerate(ACHUNKS):
            wt = lead if k % 2 == 0 else follow
            # QK matmuls, row-packed pairs: block at rows 0:32 and 64:96 run
            # concurrently on disjoint PE row groups.
            h1 = min(w, 512)
            nc.tensor.matmul(
                wt[:pn, 0:h1],
                self.qt[0:HD, i * 128 : i * 128 + pn],
                self.kt[0:HD, off : off + h1],
                start=True,
                stop=True,
            )
            if w > 512:
                h2 = w - 512
                nc.tensor.matmul(
                    wt[:pn, 512 : 512 + h2],
                    self.qt[64 : 64 + HD, i * 128 : i * 128 + pn],
                    self.kt[64 : 64 + HD, off + 512 : off + w],
                    start=True,
                    stop=True,
                )
            if k < 4:
                nc.scalar.activation(
                    e_strip[:pn, off : off + w],
                    wt[:pn, 0:w],
                    AF.Exp,
                    accum_out=rpart[:pn, k : k + 1],
                )
            else:
                # tail chunk: row-sum via a DVE fold instead of the ~287ns
                # ACT accumulator read
                nc.scalar.activation(
                    e_strip[:pn, off : off + w], wt[:pn, 0:w], AF.Exp
                )
            if k == 0 and self.prev is not None:
                # strip i-1 colsum into follow(i)[cols 0:38] (= lead(i-1),
                # whose last ACT read just drained), before c1 refills it.
                # Concurrent ACT (c0) is on the other buffer's banks.
                self._emit_colsum(self.first_cs, follow)
                self.first_cs = False
            if k == 2 and self.p_tiles is not None:
                emitted = 0
                while (self.pqueue and self.pqueue[0][0] <= i - 2
                       and emitted < 2):
                    self._emit_p()
                    emitted += 1

        # R, 1/R, vcr (persisted)
        tfold = P.small_pool.tile([128, MTAIL // 2], BF16, tag="tfold")
        nc.vector.tensor_add(
            tfold[:pn, :],
            e_strip[:pn, MMAIN : MMAIN + MTAIL // 2],
            e_strip[:pn, MMAIN + MTAIL // 2 : N],
        )
        nc.vector.tensor_reduce(
            rpart[:pn, 4:5], tfold[:pn, :], axis=AXL.X, op=ALU.add
        )
        r_sum = P.small_pool.tile([128, 1], F32, tag="rsum")
        nc.vector.tensor_reduce(
            r_sum[:pn, :], rpart[:pn, 0:5], axis=AXL.X, op=ALU.add
        )
        r_inv = P.small_pool.tile([128, 1], F32, tag="rinv")
        nc.vector.reciprocal(r_inv[:pn, :], r_sum[:pn, :])
        vcr = self.vcr_all[:, i * D : (i + 1) * D]
        nc.vector.tensor_scalar_mul(
            vcr[:pn, :], self.vc[:pn, i * D : (i + 1) * D], r_inv[:pn, 0:1]
        )
        # E2 = E*E: main span for the in-loop P banks, tail span persisted
        e2a = P.e2_pool.tile([128, MMAIN], BF16, tag="e2a")
        nc.vector.tensor_mul(
            e2a[:pn, :], e_strip[:pn, 0:MMAIN], e_strip[:pn, 0:MMAIN]
        )
        slot = self.tail_slot(i)
        nc.vector.tensor_mul(
            P.e2tail[:pn, slot * MTAIL : (slot + 1) * MTAIL],
            e_strip[:pn, MMAIN:N],
            e_strip[:pn, MMAIN:N],
        )
        self.pqueue.append((i, e2a, vcr, pn))
        self.prev = (e_strip, pn, lead)

    def run(self, strips):
        for i in strips:
            self.strip(i)

    def epilogue(self):
        # colsum of the last strip + the lagged P batches
        self._emit_colsum(False, self.prev[2])
        while self.pqueue:
            self._emit_p()


def _finalize(tc, P, m, tail_in_w=False):
    """Task finalize: evacuate P, deferred tail-P, transpose P windows with
    1/C folded into the evacuation copies, f accumulation, projection."""
    nc = tc.nc
    task = m.task
    # 1/C in window layout (c_sb[j, g] = C[g*128+j])
    c_inv = P.fin_pool.tile([128, NCH], F32, tag=f"cinv{task}", name=f"cinv_{task}")
    nc.vector.reciprocal(c_inv[:], m.c_sb[:])

    # deferred tail-P: P[:, 4096:4800] from persisted E2-tail slices.
    # For the last task the W banks are free, so the tail accumulates into
    # Wa immediately (PE still warm from the strips) while the P banks
    # evacuate on DVE in parallel; mid-kernel (task 0) it must reuse bank 0
    # after its evacuation since task 1's strips own the W banks.
    if tail_in_w:
        tailbank = P.wa

    # P -> SBUF (bf16)
    p_sb = P.fin_pool.tile([128, NPB * MC], BF16, tag="psb", name=f"psb_{task}")
    for pc in range(NPB):
        nc.vector.tensor_copy(
            p_sb[0:102, pc * MC : (pc + 1) * MC], m.p_tiles[pc][0:102, :]
        )

    if not tail_in_w:
        tailbank = P.ppool.tile([128, MC], F32, tag="p0", name=f"tailp_{task}")
    # start=True on the first pair zeroes the (whole) target bank rows, so
    # no memset is needed and the pending-zero region state stays uniform.
    for s in range(NCH):
        pns = _pn(s)
        slot = m.tail_slot(s)
        vcr = m.vcr_all[:, s * D : (s + 1) * D]
        nc.tensor.matmul(
            tailbank[0:D, 0:MC],
            vcr[:pns, :],
            P.e2tail[:pns, slot * MTAIL : slot * MTAIL + MC],
            start=(s == 0),
            stop=False,
            tile_position=(0, 0),
            skip_group_check=True,
        )
        nc.tensor.matmul(
            tailbank[64 : 64 + D, 0 : MTAIL - MC],
            vcr[:pns, :],
            P.e2tail[:pns, slot * MTAIL + MC : (slot + 1) * MTAIL],
            start=(s == 0),
            stop=False,
            tile_position=(0, 64),
            skip_group_check=True,
        )
    p_sbt = P.fin_pool.tile([128, MC], BF16, tag="psbt", name=f"psbt_{task}")
    nc.vector.tensor_copy(p_sbt[0:D, 0:MC], tailbank[0:D, 0:MC])
    nc.vector.tensor_copy(
        p_sbt[64 : 64 + D, 0 : MTAIL - MC], tailbank[64 : 64 + D, 0 : MTAIL - MC]
    )

    # PT: transpose P 128-m-windows -> [128m, 38] bf16 via freed P banks,
    # folding the 1/C[m] scaling into the evacuation copy.
    pt_sb = P.fin_pool.tile([128, NCH * D], BF16, tag="ptsb", name=f"ptsb_{task}")
    f_ps = P.ppool.tile([128, MC], F32, tag="p3", name=f"fps_{task}")
    for g in range(NCH):
        gw = _pn(g)
        if g * 128 < MMAIN:
            ci = (g * 128) // MC
            part = 0 if ci % 2 == 0 else 64
            col0 = (ci // 2) * MC + (g * 128) % MC
            src = p_sb[part : part + D, col0 : col0 + gw]
        else:
            mo = g * 128 - MMAIN
            part = 0 if mo < MC else 64
            src = p_sbt[part : part + D, mo % MC : mo % MC + gw]
        tp = P.ppool.tile(
            [128, MC], BF16, tag=f"p{1 + (g % 2)}", name=f"tp_{task}_{g}"
        )
        nc.tensor.transpose(
            tp[:gw, :D], src, P.ident[part : part + D, part : part + D]
        )
        nc.vector.tensor_scalar_mul(
            pt_sb[:gw, g * D : (g + 1) * D], tp[:gw, :D], c_inv[:gw, g : g + 1]
        )
        # f += pt_g^T @ vc_g (accumulated in PSUM across windows)
        nc.tensor.matmul(
            f_ps[0:D, 0:D],
            pt_sb[:gw, g * D : (g + 1) * D],
            m.vc[:gw, g * D : (g + 1) * D],
            start=(g == 0),
            stop=(g == NCH - 1),
        )
    f_sb = P.fin_pool.tile([D, D], F32, tag="fsb", name=f"fsb_{task}")
    nc.vector.tensor_copy(f_sb[:], f_ps[0:D, 0:D])

    # per-head projection slice: out_h[d, j] = sum_c f[c,d] * pwt[c, j]
    o_ps = P.ppool.tile([128, MC], F32, tag="p2", name=f"ops_{task}")
    nc.tensor.matmul(o_ps[0:D, 0:C], f_sb[:], P.pwt_sb[:], start=True, stop=True)
    o_sb = P.fin_pool.tile([D, C], F32, tag=f"osb{task}", name=f"osb_{task}")
    nc.vector.tensor_copy(o_sb[:], o_ps[0:D, 0:C])
    nc.sync.dma_start(P.out[task], o_sb[:])


def _tile_kernel(tc, out, x1, x2, wq4, wk4, wv, pwt, posb):
    nc = tc.nc
    from contextlib import ExitStack

    with ExitStack() as ctx:
        P = _Pools()
        P.out = out
        # ---------------- pools ----------------
        # PSUM: P accumulators 4 banks + Wa (2 banks) + Wb (2 banks) = 8
        P.ppool = ctx.enter_context(tc.tile_pool(name="ppsum", bufs=1, space="PSUM"))
        wapool = ctx.enter_context(tc.tile_pool(name="wapsum", bufs=1, space="PSUM"))
        wbpool = ctx.enter_context(tc.tile_pool(name="wbpsum", bufs=1, space="PSUM"))
        # SBUF pools
        const_pool = ctx.enter_context(tc.tile_pool(name="const", bufs=1))
        xt_pool = ctx.enter_context(tc.tile_pool(name="xt", bufs=1))
        qk_pool = ctx.enter_context(tc.tile_pool(name="qk", bufs=1))
        vc_pool = ctx.enter_context(tc.tile_pool(name="vc", bufs=1))
        P.e_pool = ctx.enter_context(tc.tile_pool(name="estrip", bufs=2))
        P.e2_pool = ctx.enter_context(tc.tile_pool(name="e2strip", bufs=3))
        P.small_pool = ctx.enter_context(tc.tile_pool(name="small", bufs=3))
        P.fin_pool = ctx.enter_context(tc.tile_pool(name="fin", bufs=1))

        # ---------------- constants ----------------
        P.ident = const_pool.tile([128, 128], BF16, tag="identb")
        make_identity(nc, P.ident)
        P.ones_col = const_pool.tile([128, 1], BF16, tag="ones")
        nc.vector.memset(P.ones_col[:], 1.0)

        P.z38 = const_pool.tile([128, D], BF16, tag="z38")
        nc.vector.memset(P.z38[:], 0.0)
        P.fillrhs = const_pool.tile([128, MC], BF16, tag="fillrhs")
        nc.vector.memset(P.fillrhs[:], 0.001)

        wq4_sb = const_pool.tile([128, 256], BF16, tag="wq4")
        nc.sync.dma_start(wq4_sb[:], wq4[:])
        wk4_sb = const_pool.tile([128, 256], BF16, tag="wk4")
        nc.sync.dma_start(wk4_sb[:], wk4[:])
        wv_sb = const_pool.tile([128, 2 * HD], BF16, tag="wv")
        nc.sync.dma_start(wv_sb[:], wv[:])
        P.pwt_sb = const_pool.tile([D, C], F32, tag="pwt")
        nc.sync.dma_start(P.pwt_sb[:], pwt[:])
        pos_sb = const_pool.tile([128, NCH * 6], BF16, tag="posb")
        nc.sync.dma_start(pos_sb[:], posb[:])

        # persisted E2 tail slices: 38 strip slots + NOVL spares for the
        # task-boundary overlap strips
        P.e2tail = const_pool.tile(
            [128, (NCH + NOVL) * MTAIL], BF16, tag="e2tail"
        )

        # working score PSUM tiles (double-buffered 1024-wide chunks)
        P.wa = wapool.tile([128, 1024], F32, tag="wa")
        P.wb = wbpool.tile([128, 1024], F32, tag="wb")

        # ---------------- prep: xT (DMA transpose), qT4, kT4, vc ----------------
        NMC = (N + MC - 1) // MC   # 10 column chunks of 512 (last = 192)

        def _mw(mc):
            return MC if mc < NMC - 1 else N - (NMC - 1) * MC

        xts_of = {}
        x_of = {1: x2, 0: x1}
        for t in (1, 0):
            xts_of[t] = [
                xt_pool.tile([128, N], BF16, tag=f"xt{ch}_{t}", name=f"xt{ch}_{t}")
                for ch in (0, 1)
            ]
        # x2 first (task-0's kt needs all of it before strip 0), x1-mc0 early
        # (qt0-mc0), x1 rest after; alternate the two HWDGE rings
        # (qSPDynamicHW via sync, qActDynamicHW via scalar) for 2x overlap.
        dma_order = [(1, ch, mc) for mc in range(NMC) for ch in (0, 1)]
        dma_order[4:4] = [(0, 0, 0), (0, 1, 0)]
        dma_order += [(0, ch, mc) for mc in range(1, NMC) for ch in (0, 1)]
        for di, (t, ch, mc) in enumerate(dma_order):
            mw = _mw(mc)
            eng = nc.sync if di % 2 == 0 else nc.scalar
            eng.dma_start(
                xts_of[t][ch][:, mc * MC : mc * MC + mw],
                x_of[t][mc * MC : mc * MC + mw, ch * 128 : (ch + 1) * 128],
                transpose=True,
            )

        qt_of, kt_of, vc_of = {}, {}, {}
        for t in (0, 1):
            qt_of[t] = qk_pool.tile([128, N], BF16, tag=f"qt{t}", name=f"qt{t}")
            kt_of[t] = qk_pool.tile([128, N], BF16, tag=f"kt{t}", name=f"kt{t}")
            vc = vc_pool.tile([128, NCH * D], BF16, tag=f"vc{t}", name=f"vc{t}")
            nc.vector.memset(vc[:], 0.0)
            vc_of[t] = vc

        def emit_qk_job(job, berth, evac_act=False):
            dst, w_sb, t, mc = job
            mw = _mw(mc)
            xts = xts_of[t]
            nc.tensor.matmul(
                berth[:, :mw],
                w_sb[:, 0:128],
                xts[0][:, mc * MC : mc * MC + mw],
                start=True,
                stop=False,
            )
            nc.tensor.matmul(
                berth[:, :mw],
                w_sb[:, 128:256],
                xts[1][:, mc * MC : mc * MC + mw],
                start=False,
                stop=True,
            )
            if evac_act:
                nc.scalar.copy(dst[:, mc * MC : mc * MC + mw], berth[:, :mw])
            else:
                nc.vector.tensor_copy(
                    dst[:, mc * MC : mc * MC + mw], berth[:, :mw]
                )

        def emit_vc_job(t, j, berth):
            pn = _pn(j)
            vcd = vc_of[t]
            xts = xts_of[t]
            nc.tensor.matmul(
                berth[:pn, :HD],
                xts[0][:, j * 128 : j * 128 + pn],
                wv_sb[:, 0:HD],
                start=True,
                stop=False,
            )
            nc.tensor.matmul(
                berth[:pn, :HD],
                xts[1][:, j * 128 : j * 128 + pn],
                wv_sb[:, HD : 2 * HD],
                start=False,
                stop=True,
            )
            nc.vector.tensor_copy(vcd[:pn, j * D : j * D + HD], berth[:pn, :HD])
            nc.vector.tensor_copy(
                vcd[:pn, j * D + HD : (j + 1) * D],
                pos_sb[:pn, j * 6 : (j + 1) * 6],
            )

        P.emit_qk_job = emit_qk_job
        P.emit_vc_job = emit_vc_job

        berths = [
            P.wa[:, 0:512], P.wa[:, 512:1024],
            P.wb[:, 0:512], P.wb[:, 512:1024],
        ]
        for b in range(NPB):
            pb = P.ppool.tile([128, MC], F32, tag=f"p{b}", name=f"prepb{b}")
            berths.append(pb[:, 0:512])

        # all prep upfront, task-0-critical first (kt1 chases the x2 DMAs)
        qk_jobs = (
            [(kt_of[1], wk4_sb, 1, mc) for mc in range(NMC)]
            + [(qt_of[0], wq4_sb, 0, mc) for mc in range(NMC)]
            + [(qt_of[1], wq4_sb, 1, mc) for mc in range(NMC)]
            + [(kt_of[0], wk4_sb, 0, mc) for mc in range(NMC)]
        )
        qk_jobs.insert(2, qk_jobs.pop(NMC))  # qt0-mc0 early
        for idx, job in enumerate(qk_jobs):
            emit_qk_job(job, berths[idx % len(berths)], evac_act=(idx % 2 == 1))
        for idx, (t, j) in enumerate(
            [(1, j) for j in range(NCH)] + [(0, j) for j in range(NCH)]
        ):
            emit_vc_job(t, j, berths[idx % len(berths)])

        P.trickle_qk = deque()
        P.trickle_vc = deque()

        # ---------------- main: two head-tasks, boundary-overlapped --------
        # out[0] = fundamental_2 = fundamental(q1, k2, v2)
        # out[1] = fundamental_1 = fundamental(q2, k1, v1)
        m0 = StripMachine(tc, P, 0, qt_of[0], kt_of[1], vc_of[1],
                          defer_pzero=False)
        m0.run(range(NCH))
        m0.epilogue()
        m1 = StripMachine(tc, P, 1, qt_of[1], kt_of[0], vc_of[0],
                          defer_pzero=True)
        m1.run(range(NOVL))
        _finalize(tc, P, m0)
        m1.run(range(NOVL, NCH))
        m1.epilogue()
        _finalize(tc, P, m1, tail_in_w=True)


# ---------------------------------------------------------------------------
# host side
# ---------------------------------------------------------------------------

_CACHE = {}


def _get_nc(reps: int = 1):
    key = f"nc{reps}"
    if key not in _CACHE:
        nc = bacc.Bacc(
            "TRN2", target_bir_lowering=False, debug=False, num_devices=8
        )
        build_kernel(nc, reps=reps)
        nc.compile()
        _CACHE[key] = nc
    return _CACHE[key]


def _positional_np():
    ys = np.linspace(-1.0, 1.0, H_IMG)
    xs = np.linspace(-1.0, 1.0, W_IMG)
    p3 = np.repeat(ys, W_IMG) / _FY_N
    p4 = np.tile(xs, H_IMG) / _FX_N
    pos = np.stack([p3 * p3, p4 * p4, p3 * p4, p3, p4, np.ones_like(p3)], axis=-1)
    return pos.astype(np.float32)  # [N, 6]


def _prep_inputs(x1, x2, qkv_w, proj_w):
    bf = ml_dtypes.bfloat16
    x1b = np.ascontiguousarray(x1.reshape(N, C)).astype(bf)
    x2b = np.ascontiguousarray(x2.reshape(N, C)).astype(bf)

    pos = _positional_np()
    posb = np.zeros((128, NCH * 6), np.float32)
    for j in range(NCH):
        pn = 128 if j < NCH - 1 else LASTP
        posb[:pn, j * 6 : (j + 1) * 6] = pos[j * 128 : j * 128 + pn]
    posb = posb.astype(bf)

    def wlayout(w_h):  # w_h: [rows, 256] -> lhsT halves layout [128, 2*rows_pad]
        wt = w_h.T.astype(np.float32)  # [256, rows]
        return np.concatenate([wt[0:128], wt[128:256]], axis=1)

    in_maps = []
    for h in range(H):
        wq = qkv_w[HD * h : HD * (h + 1), :] * SCALE          # [32, 256]
        wk = qkv_w[C + HD * h : C + HD * (h + 1), :]          # [32, 256]
        wv_ = qkv_w[2 * C + HD * h : 2 * C + HD * (h + 1), :]  # [32, 256]
        wq4 = np.tile(wq, (4, 1))                              # [128, 256]
        wk4 = np.tile(wk, (4, 1))
        in_maps.append(
            {
                "x1": x1b,
                "x2": x2b,
                "wq4": wlayout(wq4).astype(bf),               # [128, 256]
                "wk4": wlayout(wk4).astype(bf),
                "wv": wlayout(wv_).astype(bf),                # [128, 64]
                "pwt": np.ascontiguousarray(
                    proj_w[:, D * h : D * (h + 1)].T
                ).astype(np.float32),                          # [38, 256]
                "posb": posb,
            }
        )
    return in_maps


def run(x1, x2, qkv_w, proj_w, proj_b, trace=False, reps=1):
    nc = _get_nc(reps=reps)
    in_maps = _prep_inputs(x1, x2, qkv_w, proj_w)
    res = run_bass_kernel_spmd(nc, in_maps, list(range(H)), trace=trace)
    outs = np.stack([res.results[h]["out"] for h in range(H)])  # [8, 2, 38, 256]
    summed = outs.sum(axis=0) + proj_b[None, None, :].astype(np.float32)
    f2 = summed[0][None]  # (1, 38, 256)
    f1 = summed[1][None]
    return (f2, f1), res


def kernel(x1, x2, qkv_w, proj_w, proj_b):
    x1 = np.asarray(x1, np.float32)
    x2 = np.asarray(x2, np.float32)
    qkv_w = np.asarray(qkv_w, np.float32)
    proj_w = np.asarray(proj_w, np.float32)
    proj_b = np.asarray(proj_b, np.float32)
    (f2, f1), _ = run(x1, x2, qkv_w, proj_w, proj_b)
    return f2, f1


# revision 37
# speedup vs baseline: 1.1892x; 1.1892x over previous
"""Trainium2 Bass kernel for nn_CrossAttention_37718402794152.

Head-parallel sharding across 8 NeuronCores: core h computes head h of both
"fundamental" bilinear attention outputs (dual-softmax cross attention), plus
its per-head slice of the final projection; the host sums the 8 partial
projections and adds the bias.

Per core / head-task (q from one input, k,v from the other):
  a = (q k^T) * scale        (PE, bf16, row-packed pairs into Wa/Wb PSUM)
  E = exp(a)                 (ACT, 5 chunks/strip: 4x1024 + 704, accum -> R)
  E2 = E*E                   (DVE, split 4096 + 704-tail passes)
  C += colsum(E)             (PE: per-128-window stationary-trick matmuls)
  P[c,m] += (vc[n,c]/R[n]) E2[n,m]   (PE, col-tiled pairs, 4 PSUM banks for
                              m<4096; the 704-wide m-tail is accumulated at
                              task end from persisted E2-tail slices)
  f[c,d] = sum_m P[c,m]/C[m] vc[m,d] (PE, via P-window transposes; 1/C is
                              folded into the transpose evacuation copies)
  out_h  = f^T-projected slice       (PE)

since softmax(a,-1)*softmax(a,-2) = exp(2a) / (rowsum(exp a) * colsum(exp a)).

The scalar engine (exp) is the critical path: per strip it runs 5 ACTIVATE
instructions (4800 payload cycles + 5x~310 fixed) + 5 accumulator reads.
The task boundary is overlapped: task-1's first two strips are emitted
before task-0's finalize so the ACT pipeline never drains mid-kernel.
"""

from collections import deque

import numpy as np
import ml_dtypes

import concourse.bass as bass
import concourse.mybir as mybir
import concourse.tile as tile
from concourse import bacc
from concourse.bass_utils import run_bass_kernel_spmd
from concourse.masks import make_identity

F32 = mybir.dt.float32
BF16 = mybir.dt.bfloat16
AF = mybir.ActivationFunctionType
ALU = mybir.AluOpType
AXL = mybir.AxisListType

# Problem constants (hardcoded; kernel.py must be self-contained).
N = 4800            # tokens
C = 256             # model dim
H = 8               # heads
HD = 32             # head dim
D = HD + 6          # 38: v + 6 positional features
SCALE = HD ** -0.5
H_IMG, W_IMG = 60, 80
_FX_N = (517.0 / 9.0) / 80.0 * 2.0
_FY_N = (517.0 / 8.0) / 60.0 * 2.0

NCH = (N + 127) // 128          # 38 n-strips of 128 (last = 64)
LASTP = N - (NCH - 1) * 128     # 64
MC = 512                        # P accumulation m-chunk (one PSUM bank)
NPB = 4                         # P PSUM banks (m < 4096 pair-packed)
MMAIN = NPB * 2 * MC            # 4096
MTAIL = N - MMAIN               # 704 (deferred-tail m columns)
# score chunks per strip: (offset, width); alternate lead/follow buffers
ACHUNKS = [(0, 1024), (1024, 1024), (2048, 1024), (3072, 1024), (4096, 704)]
NOVL = 3          # task-1 strips emitted before task-0's finalize
# Zero-accumulate filler matmuls per strip: saturating the PE flips the HAM
# clock gate to 8/8 (2.4 GHz) after one cold strip and HOLDS it there, which
# halves the real PE work and leaves ACT (exp) as the only critical path.
# Fewer fillers while prep jobs still trickle through the strips.
FILL_STEADY = [3, 3, 2, 3, 2]   # after each chunk's QK
FILL_TRICKLE = [2, 2, 1, 2, 1]


def _pn(i):
    return 128 if i < NCH - 1 else LASTP


def build_kernel(nc: bass.Bass, reps: int = 1):
    x1 = nc.dram_tensor("x1", [N, C], BF16, kind="ExternalInput").ap()
    x2 = nc.dram_tensor("x2", [N, C], BF16, kind="ExternalInput").ap()
    # per-head weights, host-prepped layouts (see kernel()):
    wq4 = nc.dram_tensor("wq4", [128, 2 * 128], BF16, kind="ExternalInput").ap()
    wk4 = nc.dram_tensor("wk4", [128, 2 * 128], BF16, kind="ExternalInput").ap()
    wv = nc.dram_tensor("wv", [128, 2 * HD], BF16, kind="ExternalInput").ap()
    pwt = nc.dram_tensor("pwt", [D, C], F32, kind="ExternalInput").ap()
    posb = nc.dram_tensor("posb", [128, NCH * 6], BF16, kind="ExternalInput").ap()
    out = nc.dram_tensor("out", [2, D, C], F32, kind="ExternalOutput").ap()

    with tile.TileContext(nc) as tc:
        for _ in range(reps):
            _tile_kernel(tc, out, x1, x2, wq4, wk4, wv, pwt, posb)
    return nc


class _Pools:
    pass


class StripMachine:
    """Emits the strip loop for one head-task with lag-1 colsum and lag-2 P
    accumulation so the PE never head-of-line-blocks the score refills."""

    def __init__(self, tc, P, task, qt, kt, vc, defer_pzero):
        self.tc = tc
        self.P = P
        self.nc = tc.nc
        self.task = task
        self.qt = qt
        self.kt = kt
        self.vc = vc
        self.defer_pzero = defer_pzero
        self.pzero_done = False
        self.first_cs = True
        self.fill_rr = 0
        nc = self.nc
        # P-bank tiles are created at first use (same tags => same banks;
        # creating them in first-use order keeps the tag version chain
        # consistent with emission order across the task boundary).
        self.p_tiles = None
        if not defer_pzero:
            self._make_p_tiles()
        # column-sum accumulator: c_sb[j, g] = sum_n E[n, g*128+j]
        self.c_sb = P.fin_pool.tile(
            [128, NCH], F32, tag=f"csb{task}", name=f"csb_{task}"
        )
        # per-strip 1/R-scaled vc, persisted for the deferred tail-P
        self.vcr_all = P.fin_pool.tile(
            [128, NCH * D], BF16, tag=f"vcr{task}", name=f"vcr_{task}"
        )
        self.prev = None        # (e_strip, pn, lead) of strip i-1
        self.pqueue = []        # [(strip, e2a, vcr_slice, pn), ...] lag-2

    def _make_p_tiles(self):
        self.p_tiles = [
            self.P.ppool.tile(
                [128, MC], F32, tag=f"p{b}", name=f"p{b}_t{self.task}"
            )
            for b in range(NPB)
        ]
        for b in range(NPB):
            self.nc.vector.memset(self.p_tiles[b][:], 0.0)
        self.pzero_done = True

    def tail_slot(self, s):
        # task-1 strips 0-2 are emitted before task-0's finalize (which reads
        # slots 0..37), so they get the three spare slots.
        return s if (self.task == 0 or s >= NOVL) else NCH + s

    def _emit_colsum(self, first, target):
        # target: the W buffer NOT being ACT-read at emission time (cols 0:38
        # are refilled by the next QK chunk into it, after the evacuation).
        nc = self.nc
        e_prev, ppn, _ = self.prev
        for g in range(NCH):
            gw = _pn(g)
            nc.tensor.matmul(
                target[:gw, g : g + 1],
                e_prev[:ppn, g * 128 : g * 128 + gw],
                self.P.ones_col[:ppn, :],
                start=True,
                stop=True,
            )
        if first:
            nc.vector.tensor_copy(self.c_sb[:, :], target[:, 0:NCH])
        else:
            nc.vector.tensor_add(
                self.c_sb[:, :], self.c_sb[:, :], target[:, 0:NCH]
            )

    def _emit_fill(self, n):
        # PE-saturating zero-adds (value-neutral: lhsT is all zeros)
        nc = self.nc
        P = self.P
        if self.p_tiles is None:
            return
        for _ in range(n):
            nc.tensor.matmul(
                self.p_tiles[self.fill_rr % NPB][0:D, 0:MC],
                P.z38[:, :],
                P.fillrhs[:, :],
                start=False,
                stop=False,
                tile_position=(0, 0),
                skip_group_check=True,
            )
            self.fill_rr += 1

    def _emit_p(self):
        nc = self.nc
        _, e2a, vcr, pn = self.pqueue.pop(0)
        for pc in range(NPB):
            mc0, mc1 = 2 * pc, 2 * pc + 1
            nc.tensor.matmul(
                self.p_tiles[pc][0:D, 0:MC],
                vcr[:pn, :],
                e2a[:pn, mc0 * MC : (mc0 + 1) * MC],
                start=False,
                stop=False,
                tile_position=(0, 0),
                skip_group_check=True,
            )
            nc.tensor.matmul(
                self.p_tiles[pc][64 : 64 + D, 0:MC],
                vcr[:pn, :],
                e2a[:pn, mc1 * MC : (mc1 + 1) * MC],
                start=False,
                stop=False,
                tile_position=(0, 64),
                skip_group_check=True,
            )

    def strip(self, i):
        nc = self.nc
        P = self.P
        if self.defer_pzero and not self.pzero_done and i >= NOVL:
            self._make_p_tiles()
        pn = _pn(i)
        lead = P.wa if i % 2 == 0 else P.wb
        follow = P.wb if i % 2 == 0 else P.wa
        e_strip = P.e_pool.tile([128, N], BF16, tag="e")
        rpart = P.small_pool.tile([128, 5], F32, tag="rpart")

        for k, (off, w) in enumerate(ACHUNKS):
            wt = lead if k % 2 == 0 else follow
            # QK matmuls, row-packed pairs: block at rows 0:32 and 64:96 run
            # concurrently on disjoint PE row groups.
            h1 = min(w, 512)
            nc.tensor.matmul(
                wt[:pn, 0:h1],
                self.qt[0:HD, i * 128 : i * 128 + pn],
                self.kt[0:HD, off : off + h1],
                start=True,
                stop=True,
            )
            if w > 512:
                h2 = w - 512
                nc.tensor.matmul(
                    wt[:pn, 512 : 512 + h2],
                    self.qt[64 : 64 + HD, i * 128 : i * 128 + pn],
                    self.kt[64 : 64 + HD, off + 512 : off + w],
                    start=True,
                    stop=True,
                )
            if k < 4:
                nc.scalar.activation(
                    e_strip[:pn, off : off + w],
                    wt[:pn, 0:w],
                    AF.Exp,
                    accum_out=rpart[:pn, k : k + 1],
                )
            else:
                # tail chunk: row-sum via a DVE fold instead of the ~287ns
                # ACT accumulator read
                nc.scalar.activation(
                    e_strip[:pn, off : off + w], wt[:pn, 0:w], AF.Exp
                )
            if k == 0 and self.prev is not None:
                # strip i-1 colsum into follow(i)[cols 0:38] (= lead(i-1),
                # whose last ACT read just drained), before c1 refills it.
                # Concurrent ACT (c0) is on the other buffer's banks.
                self._emit_colsum(self.first_cs, follow)
                self.first_cs = False
            if k == 2 and self.p_tiles is not None:
                emitted = 0
                while (self.pqueue and self.pqueue[0][0] <= i - 2
                       and emitted < 2):
                    self._emit_p()
                    emitted += 1

        # R, 1/R, vcr (persisted)
        tfold = P.small_pool.tile([128, MTAIL // 2], BF16, tag="tfold")
        nc.vector.tensor_add(
            tfold[:pn, :],
            e_strip[:pn, MMAIN : MMAIN + MTAIL // 2],
            e_strip[:pn, MMAIN + MTAIL // 2 : N],
        )
        nc.vector.tensor_reduce(
            rpart[:pn, 4:5], tfold[:pn, :], axis=AXL.X, op=ALU.add
        )
        r_sum = P.small_pool.tile([128, 1], F32, tag="rsum")
        nc.vector.tensor_reduce(
            r_sum[:pn, :], rpart[:pn, 0:5], axis=AXL.X, op=ALU.add
        )
        r_inv = P.small_pool.tile([128, 1], F32, tag="rinv")
        nc.vector.reciprocal(r_inv[:pn, :], r_sum[:pn, :])
        vcr = self.vcr_all[:, i * D : (i + 1) * D]
        nc.vector.tensor_scalar_mul(
            vcr[:pn, :], self.vc[:pn, i * D : (i + 1) * D], r_inv[:pn, 0:1]
        )
        # E2 = E*E: main span for the in-loop P banks, tail span persisted
        e2a = P.e2_pool.tile([128, MMAIN], BF16, tag="e2a")
        nc.vector.tensor_mul(
            e2a[:pn, :], e_strip[:pn, 0:MMAIN], e_strip[:pn, 0:MMAIN]
        )
        slot = self.tail_slot(i)
        nc.vector.tensor_mul(
            P.e2tail[:pn, slot * MTAIL : (slot + 1) * MTAIL],
            e_strip[:pn, MMAIN:N],
            e_strip[:pn, MMAIN:N],
        )
        self.pqueue.append((i, e2a, vcr, pn))
        self.prev = (e_strip, pn, lead)

    def run(self, strips):
        for i in strips:
            self.strip(i)

    def epilogue(self):
        # colsum of the last strip + the lagged P batches
        self._emit_colsum(False, self.prev[2])
        while self.pqueue:
            self._emit_p()


def _finalize(tc, P, m, tail_in_w=False):
    """Task finalize: evacuate P, deferred tail-P, transpose P windows with
    1/C folded into the evacuation copies, f accumulation, projection."""
    nc = tc.nc
    task = m.task
    # 1/C in window layout (c_sb[j, g] = C[g*128+j])
    c_inv = P.fin_pool.tile([128, NCH], F32, tag=f"cinv{task}", name=f"cinv_{task}")
    nc.vector.reciprocal(c_inv[:], m.c_sb[:])

    # deferred tail-P: P[:, 4096:4800] from persisted E2-tail slices.
    # For the last task the W banks are free, so the tail accumulates into
    # Wa immediately (PE still warm from the strips) while the P banks
    # evacuate on DVE in parallel; mid-kernel (task 0) it must reuse bank 0
    # after its evacuation since task 1's strips own the W banks.
    if tail_in_w:
        tailbank = P.wa

    # P -> SBUF (bf16)
    p_sb = P.fin_pool.tile([128, NPB * MC], BF16, tag="psb", name=f"psb_{task}")
    for pc in range(NPB):
        nc.vector.tensor_copy(
            p_sb[0:102, pc * MC : (pc + 1) * MC], m.p_tiles[pc][0:102, :]
        )

    if not tail_in_w:
        tailbank = P.ppool.tile([128, MC], F32, tag="p0", name=f"tailp_{task}")
    # start=True on the first pair zeroes the (whole) target bank rows, so
    # no memset is needed and the pending-zero region state stays uniform.
    for s in range(NCH):
        pns = _pn(s)
        slot = m.tail_slot(s)
        vcr = m.vcr_all[:, s * D : (s + 1) * D]
        nc.tensor.matmul(
            tailbank[0:D, 0:MC],
            vcr[:pns, :],
            P.e2tail[:pns, slot * MTAIL : slot * MTAIL + MC],
            start=(s == 0),
            stop=False,
            tile_position=(0, 0),
            skip_group_check=True,
        )
        nc.tensor.matmul(
            tailbank[64 : 64 + D, 0 : MTAIL - MC],
            vcr[:pns, :],
            P.e2tail[:pns, slot * MTAIL + MC : (slot + 1) * MTAIL],
            start=(s == 0),
            stop=False,
            tile_position=(0, 64),
            skip_group_check=True,
        )
    p_sbt = P.fin_pool.tile([128, MC], BF16, tag="psbt", name=f"psbt_{task}")
    nc.vector.tensor_copy(p_sbt[0:D, 0:MC], tailbank[0:D, 0:MC])
    nc.vector.tensor_copy(
        p_sbt[64 : 64 + D, 0 : MTAIL - MC], tailbank[64 : 64 + D, 0 : MTAIL - MC]
    )

    # PT: transpose P 128-m-windows -> [128m, 38] bf16 via freed P banks,
    # folding the 1/C[m] scaling into the evacuation copy.
    pt_sb = P.fin_pool.tile([128, NCH * D], BF16, tag="ptsb", name=f"ptsb_{task}")
    f_ps = P.ppool.tile([128, MC], F32, tag="p3", name=f"fps_{task}")
    for g in range(NCH):
        gw = _pn(g)
        if g * 128 < MMAIN:
            ci = (g * 128) // MC
            part = 0 if ci % 2 == 0 else 64
            col0 = (ci // 2) * MC + (g * 128) % MC
            src = p_sb[part : part + D, col0 : col0 + gw]
        else:
            mo = g * 128 - MMAIN
            part = 0 if mo < MC else 64
            src = p_sbt[part : part + D, mo % MC : mo % MC + gw]
        tp = P.ppool.tile(
            [128, MC], BF16, tag=f"p{1 + (g % 2)}", name=f"tp_{task}_{g}"
        )
        nc.tensor.transpose(
            tp[:gw, :D], src, P.ident[part : part + D, part : part + D]
        )
        nc.vector.tensor_scalar_mul(
            pt_sb[:gw, g * D : (g + 1) * D], tp[:gw, :D], c_inv[:gw, g : g + 1]
        )
        # f += pt_g^T @ vc_g (accumulated in PSUM across windows)
        nc.tensor.matmul(
            f_ps[0:D, 0:D],
            pt_sb[:gw, g * D : (g + 1) * D],
            m.vc[:gw, g * D : (g + 1) * D],
            start=(g == 0),
            stop=(g == NCH - 1),
        )
    f_sb = P.fin_pool.tile([D, D], F32, tag="fsb", name=f"fsb_{task}")
    nc.vector.tensor_copy(f_sb[:], f_ps[0:D, 0:D])

    # per-head projection slice: out_h[d, j] = sum_c f[c,d] * pwt[c, j]
    o_ps = P.ppool.tile([128, MC], F32, tag="p2", name=f"ops_{task}")
    nc.tensor.matmul(o_ps[0:D, 0:C], f_sb[:], P.pwt_sb[:], start=True, stop=True)
    o_sb = P.fin_pool.tile([D, C], F32, tag=f"osb{task}", name=f"osb_{task}")
    nc.vector.tensor_copy(o_sb[:], o_ps[0:D, 0:C])
    nc.sync.dma_start(P.out[task], o_sb[:])


def _tile_kernel(tc, out, x1, x2, wq4, wk4, wv, pwt, posb):
    nc = tc.nc
    from contextlib import ExitStack

    with ExitStack() as ctx:
        P = _Pools()
        P.out = out
        # ---------------- pools ----------------
        # PSUM: P accumulators 4 banks + Wa (2 banks) + Wb (2 banks) = 8
        P.ppool = ctx.enter_context(tc.tile_pool(name="ppsum", bufs=1, space="PSUM"))
        wapool = ctx.enter_context(tc.tile_pool(name="wapsum", bufs=1, space="PSUM"))
        wbpool = ctx.enter_context(tc.tile_pool(name="wbpsum", bufs=1, space="PSUM"))
        # SBUF pools
        const_pool = ctx.enter_context(tc.tile_pool(name="const", bufs=1))
        xt_pool = ctx.enter_context(tc.tile_pool(name="xt", bufs=1))
        qk_pool = ctx.enter_context(tc.tile_pool(name="qk", bufs=1))
        vc_pool = ctx.enter_context(tc.tile_pool(name="vc", bufs=1))
        P.e_pool = ctx.enter_context(tc.tile_pool(name="estrip", bufs=2))
        P.e2_pool = ctx.enter_context(tc.tile_pool(name="e2strip", bufs=3))
        P.small_pool = ctx.enter_context(tc.tile_pool(name="small", bufs=3))
        P.fin_pool = ctx.enter_context(tc.tile_pool(name="fin", bufs=1))

        # ---------------- constants ----------------
        P.ident = const_pool.tile([128, 128], BF16, tag="identb")
        make_identity(nc, P.ident)
        P.ones_col = const_pool.tile([128, 1], BF16, tag="ones")
        nc.vector.memset(P.ones_col[:], 1.0)

        P.z38 = const_pool.tile([128, D], BF16, tag="z38")
        nc.vector.memset(P.z38[:], 0.0)
        P.fillrhs = const_pool.tile([128, MC], BF16, tag="fillrhs")
        nc.vector.memset(P.fillrhs[:], 0.001)

        wq4_sb = const_pool.tile([128, 256], BF16, tag="wq4")
        nc.sync.dma_start(wq4_sb[:], wq4[:])
        wk4_sb = const_pool.tile([128, 256], BF16, tag="wk4")
        nc.sync.dma_start(wk4_sb[:], wk4[:])
        wv_sb = const_pool.tile([128, 2 * HD], BF16, tag="wv")
        nc.sync.dma_start(wv_sb[:], wv[:])
        P.pwt_sb = const_pool.tile([D, C], F32, tag="pwt")
        nc.sync.dma_start(P.pwt_sb[:], pwt[:])
        pos_sb = const_pool.tile([128, NCH * 6], BF16, tag="posb")
        nc.sync.dma_start(pos_sb[:], posb[:])

        # persisted E2 tail slices: 38 strip slots + NOVL spares for the
        # task-boundary overlap strips
        P.e2tail = const_pool.tile(
            [128, (NCH + NOVL) * MTAIL], BF16, tag="e2tail"
        )

        # working score PSUM tiles (double-buffered 1024-wide chunks)
        P.wa = wapool.tile([128, 1024], F32, tag="wa")
        P.wb = wbpool.tile([128, 1024], F32, tag="wb")

        # ---------------- prep: xT (DMA transpose), qT4, kT4, vc ----------------
        NMC = (N + MC - 1) // MC   # 10 column chunks of 512 (last = 192)

        def _mw(mc):
            return MC if mc < NMC - 1 else N - (NMC - 1) * MC

        xts_of = {}
        x_of = {1: x2, 0: x1}
        for t in (1, 0):
            xts_of[t] = [
                xt_pool.tile([128, N], BF16, tag=f"xt{ch}_{t}", name=f"xt{ch}_{t}")
                for ch in (0, 1)
            ]
        # x2 first (task-0's kt needs all of it before strip 0), x1-mc0 early
        # (qt0-mc0), x1 rest after; alternate the two HWDGE rings
        # (qSPDynamicHW via sync, qActDynamicHW via scalar) for 2x overlap.
        dma_order = [(1, ch, mc) for mc in range(NMC) for ch in (0, 1)]
        dma_order[4:4] = [(0, 0, 0), (0, 1, 0)]
        dma_order += [(0, ch, mc) for mc in range(1, NMC) for ch in (0, 1)]
        for di, (t, ch, mc) in enumerate(dma_order):
            mw = _mw(mc)
            nc.sync.dma_start(
                xts_of[t][ch][:, mc * MC : mc * MC + mw],
                x_of[t][mc * MC : mc * MC + mw, ch * 128 : (ch + 1) * 128],
                transpose=True,
            )

        qt_of, kt_of, vc_of = {}, {}, {}
        for t in (0, 1):
            qt_of[t] = qk_pool.tile([128, N], BF16, tag=f"qt{t}", name=f"qt{t}")
            kt_of[t] = qk_pool.tile([128, N], BF16, tag=f"kt{t}", name=f"kt{t}")
            vc = vc_pool.tile([128, NCH * D], BF16, tag=f"vc{t}", name=f"vc{t}")
            nc.vector.memset(vc[:], 0.0)
            vc_of[t] = vc

        def emit_qk_job(job, berth, evac_act=False):
            dst, w_sb, t, mc = job
            mw = _mw(mc)
            xts = xts_of[t]
            nc.tensor.matmul(
                berth[:, :mw],
                w_sb[:, 0:128],
                xts[0][:, mc * MC : mc * MC + mw],
                start=True,
                stop=False,
            )
            nc.tensor.matmul(
                berth[:, :mw],
                w_sb[:, 128:256],
                xts[1][:, mc * MC : mc * MC + mw],
                start=False,
                stop=True,
            )
            if evac_act:
                nc.scalar.copy(dst[:, mc * MC : mc * MC + mw], berth[:, :mw])
            else:
                nc.vector.tensor_copy(
                    dst[:, mc * MC : mc * MC + mw], berth[:, :mw]
                )

        def emit_vc_job(t, j, berth):
            pn = _pn(j)
            vcd = vc_of[t]
            xts = xts_of[t]
            nc.tensor.matmul(
                berth[:pn, :HD],
                xts[0][:, j * 128 : j * 128 + pn],
                wv_sb[:, 0:HD],
                start=True,
                stop=False,
            )
            nc.tensor.matmul(
                berth[:pn, :HD],
                xts[1][:, j * 128 : j * 128 + pn],
                wv_sb[:, HD : 2 * HD],
                start=False,
                stop=True,
            )
            nc.vector.tensor_copy(vcd[:pn, j * D : j * D + HD], berth[:pn, :HD])
            nc.vector.tensor_copy(
                vcd[:pn, j * D + HD : (j + 1) * D],
                pos_sb[:pn, j * 6 : (j + 1) * 6],
            )

        P.emit_qk_job = emit_qk_job
        P.emit_vc_job = emit_vc_job

        berths = [
            P.wa[:, 0:512], P.wa[:, 512:1024],
            P.wb[:, 0:512], P.wb[:, 512:1024],
        ]
        for b in range(NPB):
            pb = P.ppool.tile([128, MC], F32, tag=f"p{b}", name=f"prepb{b}")
            berths.append(pb[:, 0:512])

        # all prep upfront, in task-0-consumption order: kt1 (strip 0 spans
        # all m), qt0-mc0, then vc1 (window j feeds strip j's vcr), then the
        # rest for task 1.
        jobs = (
            [(kt_of[1], wk4_sb, 1, mc) for mc in range(NMC)]
            + [(qt_of[0], wq4_sb, 0, 0)]
            + [("vc", 1, j) for j in range(4)]
            + [(qt_of[0], wq4_sb, 0, mc) for mc in range(1, NMC)]
            + [("vc", 1, j) for j in range(4, NCH)]
            + [(qt_of[1], wq4_sb, 1, mc) for mc in range(NMC)]
            + [(kt_of[0], wk4_sb, 0, mc) for mc in range(NMC)]
            + [("vc", 0, j) for j in range(NCH)]
        )
        for idx, job in enumerate(jobs):
            berth = berths[idx % len(berths)]
            if job[0] == "vc":
                emit_vc_job(job[1], job[2], berth)
            else:
                emit_qk_job(job, berth, evac_act=(idx % 2 == 1))

        P.trickle_qk = deque()
        P.trickle_vc = deque()

        # ---------------- main: two head-tasks, boundary-overlapped --------
        # out[0] = fundamental_2 = fundamental(q1, k2, v2)
        # out[1] = fundamental_1 = fundamental(q2, k1, v1)
        m0 = StripMachine(tc, P, 0, qt_of[0], kt_of[1], vc_of[1],
                          defer_pzero=False)
        m0.run(range(NCH))
        m0.epilogue()
        m1 = StripMachine(tc, P, 1, qt_of[1], kt_of[0], vc_of[0],
                          defer_pzero=True)
        m1.run(range(NOVL))
        _finalize(tc, P, m0)
        m1.run(range(NOVL, NCH))
        m1.epilogue()
        _finalize(tc, P, m1, tail_in_w=True)


# ---------------------------------------------------------------------------
# host side
# ---------------------------------------------------------------------------

_CACHE = {}


def _get_nc(reps: int = 1):
    key = f"nc{reps}"
    if key not in _CACHE:
        nc = bacc.Bacc(
            "TRN2", target_bir_lowering=False, debug=False, num_devices=8
        )
        build_kernel(nc, reps=reps)
        nc.compile()
        _CACHE[key] = nc
    return _CACHE[key]


def _positional_np():
    ys = np.linspace(-1.0, 1.0, H_IMG)
    xs = np.linspace(-1.0, 1.0, W_IMG)
    p3 = np.repeat(ys, W_IMG) / _FY_N
    p4 = np.tile(xs, H_IMG) / _FX_N
    pos = np.stack([p3 * p3, p4 * p4, p3 * p4, p3, p4, np.ones_like(p3)], axis=-1)
    return pos.astype(np.float32)  # [N, 6]


def _prep_inputs(x1, x2, qkv_w, proj_w):
    bf = ml_dtypes.bfloat16
    x1b = np.ascontiguousarray(x1.reshape(N, C)).astype(bf)
    x2b = np.ascontiguousarray(x2.reshape(N, C)).astype(bf)

    pos = _positional_np()
    posb = np.zeros((128, NCH * 6), np.float32)
    for j in range(NCH):
        pn = 128 if j < NCH - 1 else LASTP
        posb[:pn, j * 6 : (j + 1) * 6] = pos[j * 128 : j * 128 + pn]
    posb = posb.astype(bf)

    def wlayout(w_h):  # w_h: [rows, 256] -> lhsT halves layout [128, 2*rows_pad]
        wt = w_h.T.astype(np.float32)  # [256, rows]
        return np.concatenate([wt[0:128], wt[128:256]], axis=1)

    in_maps = []
    for h in range(H):
        wq = qkv_w[HD * h : HD * (h + 1), :] * SCALE          # [32, 256]
        wk = qkv_w[C + HD * h : C + HD * (h + 1), :]          # [32, 256]
        wv_ = qkv_w[2 * C + HD * h : 2 * C + HD * (h + 1), :]  # [32, 256]
        wq4 = np.tile(wq, (4, 1))                              # [128, 256]
        wk4 = np.tile(wk, (4, 1))
        in_maps.append(
            {
                "x1": x1b,
                "x2": x2b,
                "wq4": wlayout(wq4).astype(bf),               # [128, 256]
                "wk4": wlayout(wk4).astype(bf),
                "wv": wlayout(wv_).astype(bf),                # [128, 64]
                "pwt": np.ascontiguousarray(
                    proj_w[:, D * h : D * (h + 1)].T
                ).astype(np.float32),                          # [38, 256]
                "posb": posb,
            }
        )
    return in_maps


def run(x1, x2, qkv_w, proj_w, proj_b, trace=False, reps=1):
    nc = _get_nc(reps=reps)
    in_maps = _prep_inputs(x1, x2, qkv_w, proj_w)
    res = run_bass_kernel_spmd(nc, in_maps, list(range(H)), trace=trace)
    outs = np.stack([res.results[h]["out"] for h in range(H)])  # [8, 2, 38, 256]
    summed = outs.sum(axis=0) + proj_b[None, None, :].astype(np.float32)
    f2 = summed[0][None]  # (1, 38, 256)
    f1 = summed[1][None]
    return (f2, f1), res


def kernel(x1, x2, qkv_w, proj_w, proj_b):
    x1 = np.asarray(x1, np.float32)
    x2 = np.asarray(x2, np.float32)
    qkv_w = np.asarray(qkv_w, np.float32)
    proj_w = np.asarray(proj_w, np.float32)
    proj_b = np.asarray(proj_b, np.float32)
    (f2, f1), _ = run(x1, x2, qkv_w, proj_w, proj_b)
    return f2, f1
